# revision 20
# baseline (speedup 1.0000x reference)
"""Per-camera channel affine (color calibration) on 8 Trainium2 cores.

out[b, c] = image[b, c] * weight[camindex[b], c] + bias[camindex[b], c]

Sharding: pure data parallel over the batch dim — 2 images per core; the tiny
weight/bias tables are gathered by camindex on the host and shipped as a
[128, 2*PLANES] coefficient tile.

I/O precision: fp16 both directions (host converts) — the per-core DMA fabric
(16 SDMA engines @ ~27 GB/s -> ~435 GB/s) is the bottleneck for this pure
streaming op, so halving the bytes halves the runtime. Worst-case error is
~6e-4 of scale, far inside the 2e-2 gate.

Engine skew: traces show SDMA engine 15 (serving SBUF ports for partitions
120-127) sustains only ~23-25 GB/s here while engines 0-14 do ~26 GB/s.
HWDGE assigns a DMA's per-partition-row descriptors to engines in contiguous
index blocks of ceil(n_rows/16) starting at engine 0 (probed empirically), and
throughput requires descriptors to stay port-aligned (engine e <-> partitions
[8e, 8e+8); cross-port writes collapse throughput). The only port-aligned way
to shed load from engine 15 is a 120-row DMA, so each plane is split as
  [128, L2=6992] "all" block   -> all 16 engines, 8 descriptors each
  [120, E=1280]  "extra" block -> engines/ports 0-14 only, 8 descriptors each
so engine 15 carries ~13% fewer bytes and every engine finishes together.
Both blocks are zero-copy views of the flat plane.
"""

import numpy as np

import concourse.bacc as bacc
import concourse.bass as bass
import concourse.mybir as mybir
import concourse.tile as tile
from concourse.bass_utils import run_bass_kernel_spmd

N_CORES = 8
B, C, H, W = 16, 3, 1024, 1024
PER_CORE = B // N_CORES          # 2 images per core
PLANES = PER_CORE * C            # 6 channel-planes per core
P = 128                          # SBUF partitions
HW = H * W                       # 1,048,576 elements per plane
PX = 120                         # partitions that also carry the extra block

L2 = 6992                        # cols of the all-engines block (x128 rows)
E = (HW - P * L2) // PX          # 1280 cols of the extra block (x120 rows)
assert P * L2 + PX * E == HW and L2 % 16 == 0 and E % 16 == 0

# Column-split factor per plane: the tail planes are split finer so the
# pipeline drain (final in-DMA -> DVE -> out-DMA) is short.
PLANE_SPLITS = [1] * (PLANES - 2) + [2, 4]

_CACHE: dict = {}


def _build_nc() -> bass.Bass:
    f16 = mybir.dt.float16
    f32 = mybir.dt.float32
    nc = bacc.Bacc()
    inA = nc.declare_dram_parameter("inA", [PLANES, P, L2], f16, isOutput=False)
    inX = nc.declare_dram_parameter("inX", [PLANES, PX, E], f16, isOutput=False)
    coef = nc.declare_dram_parameter("coef", [P, 2 * PLANES], f32, isOutput=False)
    outA = nc.declare_dram_parameter("outA", [PLANES, P, L2], f16, isOutput=True)
    outX = nc.declare_dram_parameter("outX", [PLANES, PX, E], f16, isOutput=True)

    with tile.TileContext(nc) as tc:
        with (
            tc.tile_pool(name="cpool", bufs=1) as cpool,
            tc.tile_pool(name="io", bufs=1) as io_pool,
        ):
            # coef rides the scalar (output) ring, which is idle at startup,
            # so the sync ring's first dispatch is the first image tile.
            coef_sb = cpool.tile([P, 2 * PLANES], f32)
            nc.scalar.dma_start(out=coef_sb[:], in_=coef[:])
            # Absorb the coef-DMA wait into a throwaway DVE copy so the
            # tensor_scalars below wait only on their own input DMA.
            warm = cpool.tile([P, 2 * PLANES], f32)
            nc.vector.tensor_copy(warm[:], coef_sb[:])

            def affine(region, q, np_=P):
                nc.vector.tensor_scalar(
                    region,
                    region,
                    coef_sb[0:np_, q : q + 1],
                    coef_sb[0:np_, PLANES + q : PLANES + q + 1],
                    mybir.AluOpType.mult,
                    mybir.AluOpType.add,
                )

            for q in range(PLANES):
                t = io_pool.tile([P, L2 + E], f16, tag=f"t{q}")
                ns = PLANE_SPLITS[q]
                xs = min(ns, 2)  # extra-block chunks are small already
                # Plane 0's input rides SWDGE (gpsimd): its Q7 CounterMachine
                # emits descriptors to all 16 engine lanes in parallel, so
                # every engine starts ~immediately, while the sync-ring HWDGE
                # serially generates plane 1's 128 descriptors (~5 µs ramp).
                in_eng = nc.gpsimd if q == 0 else nc.sync
                for s in range(ns):
                    a0, a1 = s * L2 // ns, (s + 1) * L2 // ns
                    in_eng.dma_start(out=t[:, a0:a1], in_=inA[q, :, a0:a1])
                    affine(t[:, a0:a1], q)
                    nc.scalar.dma_start(out=outA[q, :, a0:a1], in_=t[:, a0:a1])
                for s in range(xs):
                    x0, x1 = L2 + s * E // xs, L2 + (s + 1) * E // xs
                    nc.sync.dma_start(
                        out=t[0:PX, x0:x1], in_=inX[q, :, x0 - L2 : x1 - L2]
                    )
                    affine(t[0:PX, x0:x1], q, np_=PX)
                    nc.scalar.dma_start(
                        out=outX[q, :, x0 - L2 : x1 - L2], in_=t[0:PX, x0:x1]
                    )
    nc.compile()
    return nc


def _get_nc() -> bass.Bass:
    if "nc" not in _CACHE:
        _CACHE["nc"] = _build_nc()
    return _CACHE["nc"]


def _make_in_maps(image: np.ndarray, w: np.ndarray, b: np.ndarray):
    in_maps = []
    for i in range(N_CORES):
        sl = slice(i * PER_CORE, (i + 1) * PER_CORE)
        img16 = image[sl].reshape(PLANES, HW).astype(np.float16)
        coef = np.empty((P, 2 * PLANES), np.float32)
        coef[:, :PLANES] = w[sl].reshape(-1)[None, :]
        coef[:, PLANES:] = b[sl].reshape(-1)[None, :]
        in_maps.append(
            {
                "inA": img16[:, : P * L2].reshape(PLANES, P, L2),
                "inX": img16[:, P * L2 :].reshape(PLANES, PX, E),
                "coef": coef,
            }
        )
    return in_maps


def kernel(image, camindex, weight, bias) -> np.ndarray:
    image = np.asarray(image, dtype=np.float32)
    idx = np.asarray(camindex).astype(np.int64)
    w = np.asarray(weight, dtype=np.float32)[idx]  # [B, C]
    b = np.asarray(bias, dtype=np.float32)[idx]    # [B, C]

    nc = _get_nc()
    in_maps = _make_in_maps(image, w, b)
    res = run_bass_kernel_spmd(nc, in_maps, core_ids=list(range(N_CORES))).results
    shards = []
    for r in res:
        flat = np.concatenate(
            [r["outA"].reshape(PLANES, -1), r["outX"].reshape(PLANES, -1)], axis=1
        )
        shards.append(flat.astype(np.float32).reshape(PER_CORE, C, H, W))
    return np.concatenate(shards, axis=0)


# revision 21
# speedup vs baseline: 1.0215x; 1.0215x over previous
"""Per-camera channel affine (color calibration) on 8 Trainium2 cores.

out[b, c] = image[b, c] * weight[camindex[b], c] + bias[camindex[b], c]

Sharding: pure data parallel over the batch dim — 2 images per core; the tiny
weight/bias tables are gathered by camindex on the host and shipped as a
[128, 2*PLANES] coefficient tile.

I/O precision: fp16 both directions (host converts) — the per-core DMA fabric
(16 SDMA engines @ ~27 GB/s -> ~435 GB/s) is the bottleneck for this pure
streaming op, so halving the bytes halves the runtime. Worst-case error is
~6e-4 of scale, far inside the 2e-2 gate.

Engine skew: traces show SDMA engine 15 (serving SBUF ports for partitions
120-127) sustains only ~23-25 GB/s here while engines 0-14 do ~26 GB/s.
HWDGE assigns a DMA's per-partition-row descriptors to engines in contiguous
index blocks of ceil(n_rows/16) starting at engine 0 (probed empirically), and
throughput requires descriptors to stay port-aligned (engine e <-> partitions
[8e, 8e+8); cross-port writes collapse throughput). The only port-aligned way
to shed load from engine 15 is a 120-row DMA, so each plane is split as
  [128, L2=6992] "all" block   -> all 16 engines, 8 descriptors each
  [120, E=1280]  "extra" block -> engines/ports 0-14 only, 8 descriptors each
so engine 15 carries ~13% fewer bytes and every engine finishes together.
Both blocks are zero-copy views of the flat plane.
"""

import numpy as np

import concourse.bacc as bacc
import concourse.bass as bass
import concourse.mybir as mybir
import concourse.tile as tile
from concourse.bass_utils import run_bass_kernel_spmd

N_CORES = 8
B, C, H, W = 16, 3, 1024, 1024
PER_CORE = B // N_CORES          # 2 images per core
PLANES = PER_CORE * C            # 6 channel-planes per core
P = 128                          # SBUF partitions
HW = H * W                       # 1,048,576 elements per plane
PX = 120                         # partitions that also carry the extra block

L2 = 6992                        # cols of the all-engines block (x128 rows)
E = (HW - P * L2) // PX          # 1280 cols of the extra block (x120 rows)
assert P * L2 + PX * E == HW and L2 % 16 == 0 and E % 16 == 0

# Column-split factor per plane: the tail planes are split finer so the
# pipeline drain (final in-DMA -> DVE -> out-DMA) is short.
PLANE_SPLITS = [1] * (PLANES - 2) + [2, 4]

_CACHE: dict = {}


def _build_nc() -> bass.Bass:
    f16 = mybir.dt.float16
    f32 = mybir.dt.float32
    nc = bacc.Bacc()
    inA = nc.declare_dram_parameter("inA", [PLANES, P, L2], f16, isOutput=False)
    inX = nc.declare_dram_parameter("inX", [PLANES, PX, E], f16, isOutput=False)
    coef = nc.declare_dram_parameter("coef", [P, 2 * PLANES], f32, isOutput=False)
    outA = nc.declare_dram_parameter("outA", [PLANES, P, L2], f16, isOutput=True)
    outX = nc.declare_dram_parameter("outX", [PLANES, PX, E], f16, isOutput=True)

    with tile.TileContext(nc) as tc:
        with (
            tc.tile_pool(name="cpool", bufs=1) as cpool,
            tc.tile_pool(name="io", bufs=1) as io_pool,
        ):
            # coef rides the scalar (output) ring, which is idle at startup,
            # so the sync ring's first dispatch is the first image tile.
            coef_sb = cpool.tile([P, 2 * PLANES], f32)
            nc.scalar.dma_start(out=coef_sb[:], in_=coef[:])
            # Absorb the coef-DMA wait into a throwaway DVE copy so the
            # tensor_scalars below wait only on their own input DMA.
            warm = cpool.tile([P, 2 * PLANES], f32)
            nc.vector.tensor_copy(warm[:], coef_sb[:])

            def affine(region, q, np_=P):
                nc.vector.tensor_scalar(
                    region,
                    region,
                    coef_sb[0:np_, q : q + 1],
                    coef_sb[0:np_, PLANES + q : PLANES + q + 1],
                    mybir.AluOpType.mult,
                    mybir.AluOpType.add,
                )

            for q in range(PLANES):
                t = io_pool.tile([P, L2 + E], f16, tag=f"t{q}")
                ns = PLANE_SPLITS[q]
                xs = min(ns, 2)  # extra-block chunks are small already
                for s in range(ns):
                    a0, a1 = s * L2 // ns, (s + 1) * L2 // ns
                    nc.sync.dma_start(out=t[:, a0:a1], in_=inA[q, :, a0:a1])
                    affine(t[:, a0:a1], q)
                    nc.scalar.dma_start(out=outA[q, :, a0:a1], in_=t[:, a0:a1])
                for s in range(xs):
                    x0, x1 = L2 + s * E // xs, L2 + (s + 1) * E // xs
                    nc.sync.dma_start(
                        out=t[0:PX, x0:x1], in_=inX[q, :, x0 - L2 : x1 - L2]
                    )
                    affine(t[0:PX, x0:x1], q, np_=PX)
                    nc.scalar.dma_start(
                        out=outX[q, :, x0 - L2 : x1 - L2], in_=t[0:PX, x0:x1]
                    )
    nc.compile()
    return nc


def _get_nc() -> bass.Bass:
    if "nc" not in _CACHE:
        _CACHE["nc"] = _build_nc()
    return _CACHE["nc"]


def _make_in_maps(image: np.ndarray, w: np.ndarray, b: np.ndarray):
    in_maps = []
    for i in range(N_CORES):
        sl = slice(i * PER_CORE, (i + 1) * PER_CORE)
        img16 = image[sl].reshape(PLANES, HW).astype(np.float16)
        coef = np.empty((P, 2 * PLANES), np.float32)
        coef[:, :PLANES] = w[sl].reshape(-1)[None, :]
        coef[:, PLANES:] = b[sl].reshape(-1)[None, :]
        in_maps.append(
            {
                "inA": img16[:, : P * L2].reshape(PLANES, P, L2),
                "inX": img16[:, P * L2 :].reshape(PLANES, PX, E),
                "coef": coef,
            }
        )
    return in_maps


def kernel(image, camindex, weight, bias) -> np.ndarray:
    image = np.asarray(image, dtype=np.float32)
    idx = np.asarray(camindex).astype(np.int64)
    w = np.asarray(weight, dtype=np.float32)[idx]  # [B, C]
    b = np.asarray(bias, dtype=np.float32)[idx]    # [B, C]

    nc = _get_nc()
    in_maps = _make_in_maps(image, w, b)
    res = run_bass_kernel_spmd(nc, in_maps, core_ids=list(range(N_CORES))).results
    shards = []
    for r in res:
        flat = np.concatenate(
            [r["outA"].reshape(PLANES, -1), r["outX"].reshape(PLANES, -1)], axis=1
        )
        shards.append(flat.astype(np.float32).reshape(PER_CORE, C, H, W))
    return np.concatenate(shards, axis=0)


# revision 23
# speedup vs baseline: 1.0624x; 1.0400x over previous
"""Per-camera channel affine (color calibration) on 8 Trainium2 cores.

out[b, c] = image[b, c] * weight[camindex[b], c] + bias[camindex[b], c]

Sharding: pure data parallel over the batch dim — 2 images per core; the tiny
weight/bias tables are gathered by camindex on the host and shipped as a
[128, 2*PLANES] coefficient tile.

I/O precision: fp16 both directions (host converts) — the per-core DMA fabric
(16 SDMA engines @ ~27 GB/s -> ~435 GB/s) is the bottleneck for this pure
streaming op, so halving the bytes halves the runtime. Worst-case error is
~6e-4 of scale, far inside the 2e-2 gate.

Engine skew: traces show SDMA engine 15 (serving SBUF ports for partitions
120-127) sustains only ~23-25 GB/s here while engines 0-14 do ~26 GB/s.
HWDGE assigns a DMA's per-partition-row descriptors to engines in contiguous
index blocks of ceil(n_rows/16) starting at engine 0 (probed empirically), and
throughput requires descriptors to stay port-aligned (engine e <-> partitions
[8e, 8e+8); cross-port writes collapse throughput). The only port-aligned way
to shed load from engine 15 is a 120-row DMA, so each plane is split as
  [128, L2=6992] "all" block   -> all 16 engines, 8 descriptors each
  [120, E=1280]  "extra" block -> engines/ports 0-14 only, 8 descriptors each
so engine 15 carries ~13% fewer bytes and every engine finishes together.
Both blocks are zero-copy views of the flat plane.
"""

import numpy as np

import concourse.bacc as bacc
import concourse.bass as bass
import concourse.mybir as mybir
import concourse.tile as tile
from concourse.bass_utils import run_bass_kernel_spmd

N_CORES = 8
B, C, H, W = 16, 3, 1024, 1024
PER_CORE = B // N_CORES          # 2 images per core
PLANES = PER_CORE * C            # 6 channel-planes per core
P = 128                          # SBUF partitions
HW = H * W                       # 1,048,576 elements per plane
PX = 120                         # partitions that also carry the extra block

L2 = 6992                        # cols of the all-engines block (x128 rows)
E = (HW - P * L2) // PX          # 1280 cols of the extra block (x120 rows)
assert P * L2 + PX * E == HW and L2 % 16 == 0 and E % 16 == 0

# Column-split factor per plane: the last plane is split in half so the
# pipeline drain (final in-DMA -> DVE -> out-DMA) is short.
PLANE_SPLITS = [1] * (PLANES - 1) + [2]

_CACHE: dict = {}


def _build_nc() -> bass.Bass:
    f16 = mybir.dt.float16
    f32 = mybir.dt.float32
    nc = bacc.Bacc()
    inA = nc.declare_dram_parameter("inA", [PLANES, P, L2], f16, isOutput=False)
    inX = nc.declare_dram_parameter("inX", [PLANES, PX, E], f16, isOutput=False)
    coef = nc.declare_dram_parameter("coef", [P, 2 * PLANES], f32, isOutput=False)
    outA = nc.declare_dram_parameter("outA", [PLANES, P, L2], f16, isOutput=True)
    outX = nc.declare_dram_parameter("outX", [PLANES, PX, E], f16, isOutput=True)

    with tile.TileContext(nc) as tc:
        with (
            tc.tile_pool(name="cpool", bufs=1) as cpool,
            tc.tile_pool(name="io", bufs=1) as io_pool,
        ):
            # coef rides the scalar (output) ring, which is idle at startup,
            # so the sync ring's first dispatch is the first image tile.
            coef_sb = cpool.tile([P, 2 * PLANES], f32)
            nc.scalar.dma_start(out=coef_sb[:], in_=coef[:])
            # Absorb the coef-DMA wait into a throwaway DVE copy so the
            # tensor_scalars below wait only on their own input DMA.
            warm = cpool.tile([P, 2 * PLANES], f32)
            nc.vector.tensor_copy(warm[:], coef_sb[:])

            def affine(region, q, np_=P):
                nc.vector.tensor_scalar(
                    region,
                    region,
                    coef_sb[0:np_, q : q + 1],
                    coef_sb[0:np_, PLANES + q : PLANES + q + 1],
                    mybir.AluOpType.mult,
                    mybir.AluOpType.add,
                )

            for q in range(PLANES):
                t = io_pool.tile([P, L2 + E], f16, tag=f"t{q}")
                ns = PLANE_SPLITS[q]
                for s in range(ns):
                    a0, a1 = s * L2 // ns, (s + 1) * L2 // ns
                    x0, x1 = L2 + s * E // ns, L2 + (s + 1) * E // ns
                    nc.sync.dma_start(out=t[:, a0:a1], in_=inA[q, :, a0:a1])
                    affine(t[:, a0:a1], q)
                    nc.scalar.dma_start(out=outA[q, :, a0:a1], in_=t[:, a0:a1])
                    nc.sync.dma_start(
                        out=t[0:PX, x0:x1], in_=inX[q, :, x0 - L2 : x1 - L2]
                    )
                    affine(t[0:PX, x0:x1], q, np_=PX)
                    nc.scalar.dma_start(
                        out=outX[q, :, x0 - L2 : x1 - L2], in_=t[0:PX, x0:x1]
                    )
    nc.compile()
    return nc


def _get_nc() -> bass.Bass:
    if "nc" not in _CACHE:
        _CACHE["nc"] = _build_nc()
    return _CACHE["nc"]


def _make_in_maps(image: np.ndarray, w: np.ndarray, b: np.ndarray):
    in_maps = []
    for i in range(N_CORES):
        sl = slice(i * PER_CORE, (i + 1) * PER_CORE)
        img16 = image[sl].reshape(PLANES, HW).astype(np.float16)
        coef = np.empty((P, 2 * PLANES), np.float32)
        coef[:, :PLANES] = w[sl].reshape(-1)[None, :]
        coef[:, PLANES:] = b[sl].reshape(-1)[None, :]
        in_maps.append(
            {
                "inA": img16[:, : P * L2].reshape(PLANES, P, L2),
                "inX": img16[:, P * L2 :].reshape(PLANES, PX, E),
                "coef": coef,
            }
        )
    return in_maps


def kernel(image, camindex, weight, bias) -> np.ndarray:
    image = np.asarray(image, dtype=np.float32)
    idx = np.asarray(camindex).astype(np.int64)
    w = np.asarray(weight, dtype=np.float32)[idx]  # [B, C]
    b = np.asarray(bias, dtype=np.float32)[idx]    # [B, C]

    nc = _get_nc()
    in_maps = _make_in_maps(image, w, b)
    res = run_bass_kernel_spmd(nc, in_maps, core_ids=list(range(N_CORES))).results
    shards = []
    for r in res:
        flat = np.concatenate(
            [r["outA"].reshape(PLANES, -1), r["outX"].reshape(PLANES, -1)], axis=1
        )
        shards.append(flat.astype(np.float32).reshape(PER_CORE, C, H, W))
    return np.concatenate(shards, axis=0)
